# revision 20
# baseline (speedup 1.0000x reference)
"""Trainium2 Bass kernel for nn_MemKDMClassModel (retrieval_knn).

Computation (per sample b, fully data-parallel over the batch):
    d2[b,i]   = ||x_enc[b] - x_neigh[b,i]||^2
    w[b,i]    = exp(-d2[b,i] / sigma^2)          (= k^2 with k the RBF kernel)
    probs[b,c]= sum_i w[b,i]*onehot(y[b,i])[c] / (sum_i w[b,i] + EPS)

Sharding: pure data parallel - batch split across 8 NeuronCores.

x_neigh is streamed as fp16 (host-side downcast inside kernel(); products of
fp16 values are exact in f32 and all reductions accumulate in f32, so the
result stays well inside the 2e-2 tolerance), halving the HBM traffic that
dominates this memory-bound problem.

Per 128-sample block each of the 128 components needs ||x-n||^2. Engine
constraints: GPSIMD can't touch PSUM and has no TensorScalarPtr; DVE can't
read one PSUM tile twice in an op; so PSUM square-accums only run on ACT.
Four component paths spread the work over every engine:

  "G" (whole 8-comp groups): the group is DMA-loaded TRANSPOSED (HW xbar,
      ~23% DMA premium) as n_T [enc,sample] chunks; PE accumulates the Gram
      matrix G = n_T'.n_T + (-2 x_T)'.n_T over 4 enc-chunks; diag(G) =
      n2 - 2*x.n, extracted by one DVE STT against the identity into a d2
      column; a per-group tensor_scalar adds x2. This moves the square+
      reduce onto the otherwise idle PE.
  "A": PE fp16 identity-matmul diff -> PSUM; ACT Square in-place with
      accum_out -> d2 column.
  "Q": Pool f16 tensor_tensor diff -> SBUF; DVE STT square-accum.
  "V": DVE f16 diff; DVE STT square-accum.

  w = exp(-d2/sigma^2) on ACT per half-block (accum_out = row sum).
  scatter: one DVE STT per class (probs[:,c] = sum_i (y==c)*w_i), queued and
  drained a few per streaming group so it never bursts; each block's scatter
  overlaps the next block's stream.
"""

import numpy as np

BS, N_COMP, ENC, DIM_Y = 4096, 128, 512, 100
EPS = 1e-10
N_CORES = 8
BS_L = BS // N_CORES          # 512 samples per core
BLK = 128                     # samples per block (partition dim)
NBLK = BS_L // BLK            # 4 blocks per core
G = 8                         # comps per DMA transfer (1 MiB each in fp16)
NG = N_COMP // G              # 16 DMA groups per block
NCH = ENC // 128              # 128-row chunks per comp (gram contraction)
HALF = N_COMP // 2

N_GGRP = 2                    # groups loaded transposed (gram path)
N_A = 68                      # "A" comps (PE diff + ACT psum sqacc)
N_QA = 0                      # "Q" comps with ACT sbuf sqacc
N_QV = 44                     # "Q" comps with DVE sqacc
# remaining D-comps are "V"
PER = 9                       # queued scatter ops drained per streaming group

_CACHE: dict = {}


def _build_layout():
    """Spread G-groups among the 16 groups; assign per-comp paths in the
    remaining (diff) groups."""
    ggrp = set()
    for k in range(N_GGRP):
        ggrp.add((k * NG + NG // 2) // max(N_GGRP, 1) % NG)
    assert len(ggrp) == N_GGRP
    d_comps = [g * G + j for g in range(NG) if g not in ggrp
               for j in range(G)]
    nd = len(d_comps)
    quota = {"A": N_A, "2": N_QA, "Q": N_QV,
             "V": nd - N_A - N_QA - N_QV}
    assert quota["V"] >= 0
    acc = {k: 0 for k in quota}
    path = {}
    for t, i in enumerate(d_comps):
        k = max(quota, key=lambda q: quota[q] * (t + 1) / nd - acc[q])
        path[i] = k
        acc[k] += 1
    return ggrp, path


GGRPS, DPATH = _build_layout()


def _build_nc():
    import concourse.bacc as bacc
    import concourse.tile as tile
    import concourse.mybir as mybir
    from concourse import bass

    f32 = mybir.dt.float32
    f16 = mybir.dt.float16
    AF = mybir.ActivationFunctionType
    ALU = mybir.AluOpType

    nc = bacc.Bacc("TRN2", target_bir_lowering=False, debug=False,
                   num_devices=N_CORES)

    x_dram = nc.dram_tensor("x_enc", [BS_L, ENC], f16, kind="ExternalInput")
    n_dram = nc.dram_tensor("x_neigh", [BS_L, N_COMP, ENC], f16,
                            kind="ExternalInput")
    s_dram = nc.dram_tensor("sigma", [1, 1], f32, kind="ExternalInput")
    y_dram = nc.dram_tensor("y_neigh", [BS_L, N_COMP], f16,
                            kind="ExternalInput")
    eye_dram = nc.dram_tensor("eye", [128, 128], f16, kind="ExternalInput")
    eyen_dram = nc.dram_tensor("eyen", [128, 128], f16, kind="ExternalInput")
    out_dram = nc.dram_tensor("out", [BS_L, DIM_Y], f32,
                              kind="ExternalOutput")

    with tile.TileContext(nc) as tc:
        with (
            tc.tile_pool(name="const", bufs=1) as constp,
            tc.tile_pool(name="neigh", bufs=8) as neighp,
            tc.tile_pool(name="neiT", bufs=4) as neiTp,
            tc.tile_pool(name="xp", bufs=2) as xp,
            tc.tile_pool(name="small", bufs=2) as smallp,
            tc.tile_pool(name="diffq", bufs=6) as diffq,
            tc.tile_pool(name="diffv", bufs=3) as diffv,
            tc.tile_pool(name="junkv", bufs=3) as junkv,
            tc.tile_pool(name="junkg", bufs=3) as junkg,
            tc.tile_pool(name="scatj", bufs=3) as scatj,
            tc.tile_pool(name="outp", bufs=2) as outp,
            tc.tile_pool(name="pdiff", bufs=4, space=bass.MemorySpace.PSUM) as pdiff,
            tc.tile_pool(name="pgram", bufs=4, space=bass.MemorySpace.PSUM) as pgram,
        ):
            # ---- constants ----
            eye16 = constp.tile([128, 128], f16)
            nc.sync.dma_start(eye16[:], eye_dram[:])
            eyen16 = constp.tile([128, 128], f16)
            nc.sync.dma_start(eyen16[:], eyen_dram[:])

            # ---- cvec = -1/sigma^2 broadcast to [128, 1] ----
            sig = constp.tile([1, 1], f32)
            nc.sync.dma_start(sig[:], s_dram[:])
            sig2 = constp.tile([1, 1], f32)
            nc.vector.tensor_scalar(sig2[:], sig[:], sig[0:1, 0:1], None,
                                    op0=ALU.mult)
            rsig2 = constp.tile([1, 1], f32)
            nc.vector.reciprocal(rsig2[:], sig2[:])
            nrsig2 = constp.tile([1, 1], f32)
            nc.vector.tensor_scalar_mul(nrsig2[:], rsig2[:], -1.0)
            ones_row = constp.tile([1, 128], f32)
            nc.vector.memset(ones_row[:], 1.0)
            cvec_ps = pdiff.tile([128, 512], f32, tag="pd")
            nc.tensor.matmul(cvec_ps[:, 0:1], ones_row[:], nrsig2[:],
                             start=True, stop=True)
            cvec = constp.tile([128, 1], f32)
            nc.vector.tensor_copy(cvec[:], cvec_ps[:, 0:1])

            # scatter items pending issue on DVE: (acc_col, y_t, w_t, lo, hi, c)
            pending = []

            def scatter_drain(k):
                for _ in range(min(k, len(pending))):
                    acc_col, y_t, w_t, lo, hi, c = pending.pop(0)
                    junk = scatj.tile([BLK, hi - lo], f16, tag="sj")
                    nc.vector.scalar_tensor_tensor(
                        junk[:], y_t[:, lo:hi], float(c), w_t[:, lo:hi],
                        op0=ALU.is_equal, op1=ALU.mult, accum_out=acc_col)

            def block_finish(st):
                rowsum = smallp.tile([BLK, 1], f32, tag="rs")
                nc.vector.tensor_tensor(rowsum[:], st["rsA"][:], st["rsB"][:],
                                        op=ALU.add)
                rs_eps = smallp.tile([BLK, 1], f32, tag="rse")
                nc.vector.tensor_scalar_add(rs_eps[:], rowsum[:], EPS)
                rinv = smallp.tile([BLK, 1], f32, tag="rinv")
                nc.vector.reciprocal(rinv[:], rs_eps[:])
                out_sb = outp.tile([BLK, DIM_Y], f32, tag="out")
                nc.scalar.activation(out_sb[:], st["pA"][:], AF.Copy,
                                     scale=rinv[:, 0:1])
                nc.gpsimd.dma_start(out_dram[st["s0"]:st["s0"] + BLK, :],
                                    out_sb[:])

            prev = None
            for b in range(NBLK):
                last = b == NBLK - 1
                s0 = b * BLK
                # ---- per-block inputs ----
                x16 = xp.tile([BLK, ENC], f16, tag="x")
                nc.sync.dma_start(x16[:], x_dram[s0:s0 + BLK, :])
                n2xT = None
                x2 = None
                if N_GGRP:
                    xT = xp.tile([128, NCH, 128], f16, tag="xT")
                    nc.sync.dma_start_transpose(xT[:],
                                                x_dram[s0:s0 + BLK, :])
                    n2xT = xp.tile([128, NCH, 128], f16, tag="n2xT")
                    nc.vector.tensor_scalar_mul(n2xT[:], xT[:], -2.0)
                    x2 = smallp.tile([BLK, 1], f32, tag="x2")
                    xsq = junkv.tile([BLK, ENC], f32, tag="xsq")
                    nc.scalar.activation(xsq[:], x16[:], AF.Square,
                                         accum_out=x2[:, 0:1])
                y16 = smallp.tile([BLK, N_COMP], f16, tag="y")
                nc.sync.dma_start(y16[:], y_dram[s0:s0 + BLK, :])

                d2 = smallp.tile([BLK, N_COMP], f32, tag="d2")
                w16 = smallp.tile([BLK, N_COMP], f16, tag="w")
                rsA = smallp.tile([BLK, 1], f32, tag="rsA")
                rsB = smallp.tile([BLK, 1], f32, tag="rsB")
                probsA = outp.tile([BLK, DIM_Y], f32, tag="pA")
                probsB = outp.tile([BLK, DIM_Y], f32, tag="pB")
                st = {"s0": s0, "rsA": rsA, "rsB": rsB, "pA": probsA,
                      "pB": probsB}

                gram_bank = [None]
                gslot = [0]
                diagq = []  # pending diag extractions: (gram_slice, comp)

                def diag_drain(k):
                    for _ in range(min(k, len(diagq))):
                        gram_sl, i = diagq.pop(0)
                        jg = junkg.tile([128, 128], f32, tag="jg")
                        nc.vector.scalar_tensor_tensor(
                            jg[:], gram_sl, 1.0, eye16[:],
                            op0=ALU.mult, op1=ALU.mult,
                            accum_out=d2[:, i:i + 1])
                        if i % G == G - 1:
                            # group's diagonals done: d2[group cols] += x2
                            g0 = i - G + 1
                            nc.vector.tensor_scalar(
                                d2[:, g0:g0 + G], d2[:, g0:g0 + G],
                                x2[:, 0:1], None, op0=ALU.add)

                for g in range(NG):
                    if g in GGRPS:
                        # ---- gram path: transposed load, PE grams ----
                        nT = neiTp.tile([128, G * NCH, 128], f16, tag="nT")
                        nc.sync.dma_start_transpose(
                            nT[:], n_dram[s0:s0 + BLK, g * G:(g + 1) * G, :])
                        for j in range(G):
                            i = g * G + j
                            if gslot[0] % 4 == 0:
                                gb_tile = pgram.tile([128, 512], f32,
                                                     tag="gb")
                                gram_bank[0] = gb_tile
                            k = gslot[0] % 4
                            gslot[0] += 1
                            gram = gram_bank[0][:, k * 128:(k + 1) * 128]
                            for c in range(NCH):
                                sl = nT[:, j * NCH + c, :]
                                nc.tensor.matmul(gram, sl, sl,
                                                 start=(c == 0), stop=False)
                                nc.tensor.matmul(gram, n2xT[:, c, :], sl,
                                                 start=False,
                                                 stop=(c == NCH - 1))
                            diagq.append((gram, i))
                    else:
                        # ---- diff paths ----
                        ntile = neighp.tile([BLK, G * ENC], f16, tag="ntile")
                        nc.sync.dma_start(
                            ntile[:],
                            n_dram[s0:s0 + BLK, g * G:(g + 1) * G, :])
                        for j in range(G):
                            i = g * G + j
                            nsl = ntile[:, j * ENC:(j + 1) * ENC]
                            path = DPATH[i]
                            if path == "A":
                                dtile = pdiff.tile([BLK, ENC], f32, tag="pd")
                                nc.tensor.matmul(dtile[:], eye16[:], nsl,
                                                 start=True, stop=False)
                                nc.tensor.matmul(dtile[:], eyen16[:], x16[:],
                                                 start=False, stop=True)
                                nc.scalar.activation(
                                    dtile[:], dtile[:], AF.Square,
                                    accum_out=d2[:, i:i + 1])
                                continue
                            if path in ("Q", "2"):
                                dt16 = diffq.tile([BLK, ENC], f16, tag="dq")
                                nc.gpsimd.tensor_tensor(dt16[:], nsl, x16[:],
                                                        op=ALU.subtract)
                            else:
                                dt16 = diffv.tile([BLK, ENC], f16, tag="dv")
                                nc.vector.tensor_tensor(dt16[:], nsl, x16[:],
                                                        op=ALU.subtract)
                            if path == "2":
                                ja = junkv.tile([BLK, ENC], f32, tag="ja")
                                nc.scalar.activation(
                                    ja[:], dt16[:], AF.Square,
                                    accum_out=d2[:, i:i + 1])
                            else:
                                jv = junkv.tile([BLK, ENC], f16, tag="jv")
                                nc.vector.scalar_tensor_tensor(
                                    jv[:], dt16[:], 1.0, dt16[:],
                                    op0=ALU.mult, op1=ALU.mult,
                                    accum_out=d2[:, i:i + 1])

                    diag_drain(len(diagq)
                               if g in (NG // 2 - 1, NG - 1) else 4)
                    if g == NG // 2 - 1:
                        nc.scalar.activation(w16[:, 0:HALF], d2[:, 0:HALF],
                                             AF.Exp, scale=cvec[:, 0:1],
                                             accum_out=rsA[:, 0:1])
                        if last:
                            # shorten the tail: scatter the last block's first
                            # half while its second half still streams
                            pending.extend(
                                (probsA[:, c:c + 1], y16, w16, 0, HALF, c)
                                for c in range(DIM_Y))
                    scatter_drain(PER if not last else 2 * PER)

                # previous block's scatter fully drained over our 16 groups
                if prev is not None:
                    block_finish(prev)
                    prev = None

                # ---- block tail ----
                nc.scalar.activation(w16[:, HALF:], d2[:, HALF:],
                                     AF.Exp, scale=cvec[:, 0:1],
                                     accum_out=rsB[:, 0:1])
                if last:
                    pending.extend(
                        (probsB[:, c:c + 1], y16, w16, HALF, N_COMP, c)
                        for c in range(DIM_Y))
                else:
                    # full-width scatter, drained during the next block
                    pending.extend(
                        (probsA[:, c:c + 1], y16, w16, 0, N_COMP, c)
                        for c in range(DIM_Y))
                prev = st

            scatter_drain(len(pending))
            # combine the last block's two halves
            pAB = outp.tile([BLK, DIM_Y], f32, tag="pAB")
            nc.vector.tensor_tensor(pAB[:], prev["pA"][:], prev["pB"][:],
                                    op=ALU.add)
            prev["pA"] = pAB
            block_finish(prev)

    nc.compile()
    return nc


def _get_nc():
    if "nc" not in _CACHE:
        _CACHE["nc"] = _build_nc()
    return _CACHE["nc"]


def _get_exec():
    """Build (once) a jitted shard_map executable over 8 cores.

    Returns (fn, in_names, out_names, out_avals, n_params, mesh).
    Call as fn(*concat_inputs, *concat_zero_outputs); outputs donated.
    """
    if "exec" in _CACHE:
        return _CACHE["exec"]
    import jax
    import concourse.mybir as mybir
    from jax.sharding import Mesh, PartitionSpec
    from jax.experimental.shard_map import shard_map
    from concourse.bass2jax import (_bass_exec_p, install_neuronx_cc_hook,
                                    partition_id_tensor)

    install_neuronx_cc_hook()
    nc = _get_nc()
    partition_name = (nc.partition_id_tensor.name
                      if nc.partition_id_tensor else None)
    in_names, out_names, out_avals = [], [], []
    for alloc in nc.m.functions[0].allocations:
        if not isinstance(alloc, mybir.MemoryLocationSet):
            continue
        name = alloc.memorylocations[0].name
        if alloc.kind == "ExternalInput":
            if name != partition_name:
                in_names.append(name)
        elif alloc.kind == "ExternalOutput":
            out_names.append(name)
            out_avals.append(jax.core.ShapedArray(
                tuple(alloc.tensor_shape), mybir.dt.np(alloc.dtype)))
    n_params = len(in_names)
    all_in_names = in_names + out_names
    if partition_name is not None:
        all_in_names = all_in_names + [partition_name]
    donate = tuple(range(n_params, n_params + len(out_names)))

    def _body(*args):
        operands = list(args)
        if partition_name is not None:
            operands.append(partition_id_tensor())
        outs = _bass_exec_p.bind(
            *operands,
            out_avals=tuple(out_avals),
            in_names=tuple(all_in_names),
            out_names=tuple(out_names),
            lowering_input_output_aliases=(),
            sim_require_finite=True,
            sim_require_nnan=True,
            nc=nc,
        )
        return tuple(outs)

    devices = jax.devices()[:N_CORES]
    mesh = Mesh(np.asarray(devices), ("core",))
    specs = (PartitionSpec("core"),) * (n_params + len(out_names))
    out_specs = (PartitionSpec("core"),) * len(out_names)
    fn = jax.jit(
        shard_map(_body, mesh=mesh, in_specs=specs, out_specs=out_specs,
                  check_rep=False),
        donate_argnums=donate, keep_unused=True)
    _CACHE["exec"] = (fn, in_names, out_names, out_avals, n_params, mesh)
    return _CACHE["exec"]


def _concat_inputs(x_enc16, x_neigh16, sig, y16):
    """Per-input concatenation over cores, ordered by the NEFF's in_names."""
    eye = np.eye(128, dtype=np.float16)
    eyen = (-np.eye(128)).astype(np.float16)
    per_core = {
        "x_enc": lambda c: x_enc16[c * BS_L:(c + 1) * BS_L],
        "x_neigh": lambda c: x_neigh16[c * BS_L:(c + 1) * BS_L],
        "sigma": lambda c: sig,
        "y_neigh": lambda c: y16[c * BS_L:(c + 1) * BS_L],
        "eye": lambda c: eye,
        "eyen": lambda c: eyen,
    }
    _, in_names, _, _, _, _ = _get_exec()
    return [np.concatenate([per_core[name](c) for c in range(N_CORES)], axis=0)
            for name in in_names]


def _zero_outs():
    _, _, _, out_avals, _, _ = _get_exec()
    return [np.zeros((N_CORES * a.shape[0], *a.shape[1:]), a.dtype)
            for a in out_avals]


def kernel(x_enc, x_neigh, sigma, y_neigh):
    x_enc16 = np.ascontiguousarray(np.asarray(x_enc).astype(np.float16))
    x_neigh16 = np.ascontiguousarray(np.asarray(x_neigh).astype(np.float16))
    sig = np.ascontiguousarray(
        np.asarray(sigma).astype(np.float32).reshape(1, 1))
    y16 = np.ascontiguousarray(np.asarray(y_neigh).astype(np.float16))

    fn, in_names, out_names, out_avals, n_params, mesh = _get_exec()
    concat_in = _concat_inputs(x_enc16, x_neigh16, sig, y16)
    out_arrs = fn(*concat_in, *_zero_outs())
    oi = out_names.index("out")
    out = np.asarray(out_arrs[oi]).reshape(N_CORES, BS_L, DIM_Y)
    return out.reshape(BS, DIM_Y).astype(np.float32)


if __name__ == "__main__":
    rng = np.random.default_rng(0)
    x_enc = rng.standard_normal((BS, ENC), dtype=np.float32)
    x_neigh = rng.standard_normal((BS, N_COMP, ENC), dtype=np.float32)
    sigma = 20.0 * np.ones((1,), dtype=np.float32)  # large: exercises nonzero path
    y_neigh = rng.integers(0, DIM_Y, size=(BS, N_COMP)).astype(np.int32)
    out = kernel(x_enc=x_enc, x_neigh=x_neigh, sigma=sigma, y_neigh=y_neigh)
    # numpy oracle (mirror the fp16 rounding of the kernel inputs)
    xe = x_enc.astype(np.float16).astype(np.float32)
    xn = x_neigh.astype(np.float16).astype(np.float32)
    d2 = np.maximum(
        (xe ** 2).sum(-1)[:, None]
        + (xn ** 2).sum(-1)
        - 2.0 * np.einsum("bd,bnd->bn", xe, xn), 0.0)
    w = np.exp(-d2 / (sigma[0] ** 2))
    probs = np.zeros((BS, DIM_Y), np.float32)
    np.add.at(probs, (np.arange(BS)[:, None], y_neigh), w.astype(np.float32))
    probs /= (w.sum(-1, keepdims=True).astype(np.float32) + EPS)
    print("max abs diff:", np.abs(out - probs).max())
    print("ref max:", probs.max(), "out max:", out.max())
    print("out nonzero:", np.count_nonzero(out), "/", out.size)
